# revision 8
# baseline (speedup 1.0000x reference)
"""JPEGBase (nn_JPEGBase_240518169043) Trainium2 kernel.

The reference computes rgb->yuv, *255, blockwise 8x8 DCT, blockwise IDCT
(compress() is identity), /255, yuv->rgb.  The orthonormal DCT/IDCT pair and
the *255 / /255 cancel exactly, so the remaining math is a per-pixel 3x3
color-matrix roundtrip A = yuv2rgb @ rgb2yuv applied along the channel dim
(float32 discrepancy vs. the reference's explicit DCT roundtrip is ~1.5e-7
relative).  i_co is unused by the reference.

Sharding: pure data parallelism - batch 32 -> 4 images per core across 8
cores.  Per core the kernel streams 4 images of [3,512,512] f32 through SBUF
([128,2048] per plane), computes the three output planes as weighted sums of
the three input planes (2 DVE scalar_tensor_tensor ops + 1 ACT scale per
output plane), and streams them back.  Memory-bound: ~25 MB of HBM traffic
per core.
"""

import numpy as np
from contextlib import ExitStack

import concourse.bass as bass  # noqa: F401  (engine namespaces live on nc)
import concourse.tile as tile
from concourse import bacc, mybir
from concourse.bass_utils import run_bass_kernel_spmd

N_CORES = 8
B_FULL = 32
B_PER_CORE = B_FULL // N_CORES  # 4
C = 3
H = 512
W = 512
P = 128               # SBUF partitions
F = (H * W) // P      # 2048 floats per partition per plane


def _color_matrix():
    # kornia rgb_to_yuv / yuv_to_rgb coefficient matrices, composed in f64.
    m = np.array(
        [[0.299, 0.587, 0.114],
         [-0.147, -0.289, 0.436],
         [0.615, -0.515, -0.100]], dtype=np.float64)
    n = np.array(
        [[1.0, 0.0, 1.14],
         [1.0, -0.396, -0.581],
         [1.0, 2.029, 0.0]], dtype=np.float64)
    return n @ m


def build_nc():
    """Build + compile the per-core Bass program (same program on all cores)."""
    a = _color_matrix()
    nc = bacc.Bacc(
        "TRN2", target_bir_lowering=False, debug=False, num_devices=N_CORES
    )
    x = nc.dram_tensor(
        "x", [B_PER_CORE, C, H, W], mybir.dt.float32, kind="ExternalInput"
    ).ap()
    y = nc.dram_tensor(
        "y", [B_PER_CORE, C, H, W], mybir.dt.float32, kind="ExternalOutput"
    ).ap()
    # [b, 128, c, 2048]; partition p covers image rows [4p, 4p+4) (contiguous);
    # dim order matches the SBUF tile view [p, c, f].
    xr = x.rearrange("b c (hp hs) w -> b hp c (hs w)", hp=P)
    yr = y.rearrange("b c (hp hs) w -> b hp c (hs w)", hp=P)

    f32 = mybir.dt.float32
    HALVES = 2                  # groups per image
    F2 = F // HALVES            # free elems per plane per group
    with tile.TileContext(nc) as tc, ExitStack() as ctx:
        in_pool = ctx.enter_context(tc.tile_pool(name="in", bufs=5))
        out_pool = ctx.enter_context(tc.tile_pool(name="out", bufs=4))
        t_pool = ctx.enter_context(tc.tile_pool(name="tmp", bufs=4))

        for g in range(B_PER_CORE * HALVES):
            b, h = divmod(g, HALVES)
            fsl = slice(h * F2, (h + 1) * F2)
            # 1.5 MB transfers, half an image each.  Loads on the SP HWDGE
            # ring, stores on the ACT ring: each ring is FIFO per issuing
            # engine, so stores waiting on compute must not block loads.
            # ACT computes the *final* op per plane, so its store push never
            # waits on another engine.
            it = in_pool.tile([P, C * F2], f32)
            nc.sync.dma_start(
                it[:].rearrange("p (c f) -> p c f", c=C), xr[b][:, :, fsl]
            )
            ot = out_pool.tile([P, C * F2], f32)
            for c in range(C):
                # out_c = a[c,i]*X_i + a[c,j]*X_j + a[c,c]*X_c, diagonal term
                # largest; (i, j) = off-diagonals with |a_i| <= |a_j|:
                #   t1    = X_i * (a[c,i]/a[c,j]) + X_j     (DVE stt)
                #   t2    = t1 * (a[c,j]/a[c,c]) + X_c      (DVE stt)
                #   out_c = t2 * a[c,c]                     (ACT, single-src)
                i, j = [d for d in range(C) if d != c]
                if abs(a[c, i]) > abs(a[c, j]):
                    i, j = j, i
                sl = lambda d: slice(d * F2, (d + 1) * F2)
                t1 = t_pool.tile([P, F2], f32)
                if c == 1:
                    # Rebalance: DVE is near co-bottleneck with DMA, so route
                    # one plane's t1 through ACT (prescale) + Pool (add).
                    xs = t_pool.tile([P, F2], f32, tag="xs")
                    nc.scalar.mul(xs[:], it[:, sl(i)], float(a[c, i] / a[c, j]))
                    nc.gpsimd.tensor_tensor(
                        t1[:], xs[:], it[:, sl(j)], mybir.AluOpType.add
                    )
                else:
                    nc.vector.scalar_tensor_tensor(
                        t1[:], it[:, sl(i)], float(a[c, i] / a[c, j]),
                        it[:, sl(j)],
                        mybir.AluOpType.mult, mybir.AluOpType.add,
                    )
                t2 = t_pool.tile([P, F2], f32, tag="t2")
                nc.vector.scalar_tensor_tensor(
                    t2[:], t1[:], float(a[c, j] / a[c, c]), it[:, sl(c)],
                    mybir.AluOpType.mult, mybir.AluOpType.add,
                )
                nc.scalar.mul(ot[:, sl(c)], t2[:], float(a[c, c]))
            nc.scalar.dma_start(
                yr[b][:, :, fsl], ot[:].rearrange("p (c f) -> p c f", c=C)
            )

    nc.compile()
    return nc


_NC = None


def _get_nc():
    global _NC
    if _NC is None:
        _NC = build_nc()
    return _NC


def _in_maps(i_en):
    xs = np.ascontiguousarray(np.asarray(i_en, dtype=np.float32)).reshape(
        N_CORES, B_PER_CORE, C, H, W
    )
    return [{"x": xs[i]} for i in range(N_CORES)]


def kernel(i_co=None, i_en=None, **_):
    res = run_bass_kernel_spmd(_get_nc(), _in_maps(i_en), list(range(N_CORES)))
    return np.concatenate(
        [res.results[i]["y"] for i in range(N_CORES)], axis=0
    )


# revision 11
# speedup vs baseline: 1.0723x; 1.0723x over previous
"""JPEGBase (nn_JPEGBase_240518169043) Trainium2 kernel.

The reference computes rgb->yuv, *255, blockwise 8x8 DCT, blockwise IDCT
(compress() is identity), /255, yuv->rgb.  The orthonormal DCT/IDCT pair and
the *255 / /255 cancel exactly, so the remaining math is a per-pixel 3x3
color-matrix roundtrip A = yuv2rgb @ rgb2yuv applied along the channel dim
(float32 discrepancy vs. the reference's explicit DCT roundtrip is ~1.5e-7
relative).  i_co is unused by the reference.

Sharding: pure data parallelism - batch 32 -> 4 images per core across 8
cores.  Per core the kernel streams 4 images of [3,512,512] f32 through SBUF
([128,2048] per plane), computes the three output planes as weighted sums of
the three input planes (2 DVE scalar_tensor_tensor ops + 1 ACT scale per
output plane), and streams them back.  Memory-bound: ~25 MB of HBM traffic
per core.
"""

import numpy as np
from contextlib import ExitStack

import concourse.bass as bass  # noqa: F401  (engine namespaces live on nc)
import concourse.tile as tile
from concourse import bacc, mybir
from concourse.bass_utils import run_bass_kernel_spmd

N_CORES = 8
B_FULL = 32
B_PER_CORE = B_FULL // N_CORES  # 4
C = 3
H = 512
W = 512
P = 128               # SBUF partitions
F = (H * W) // P      # 2048 floats per partition per plane


def _color_matrix():
    # kornia rgb_to_yuv / yuv_to_rgb coefficient matrices, composed in f64.
    m = np.array(
        [[0.299, 0.587, 0.114],
         [-0.147, -0.289, 0.436],
         [0.615, -0.515, -0.100]], dtype=np.float64)
    n = np.array(
        [[1.0, 0.0, 1.14],
         [1.0, -0.396, -0.581],
         [1.0, 2.029, 0.0]], dtype=np.float64)
    return n @ m


def build_nc():
    """Build + compile the per-core Bass program (same program on all cores)."""
    a = _color_matrix()
    nc = bacc.Bacc(
        "TRN2", target_bir_lowering=False, debug=False, num_devices=N_CORES
    )
    x = nc.dram_tensor(
        "x", [B_PER_CORE, C, H, W], mybir.dt.float32, kind="ExternalInput"
    ).ap()
    y = nc.dram_tensor(
        "y", [B_PER_CORE, C, H, W], mybir.dt.float32, kind="ExternalOutput"
    ).ap()
    # [b, 128, c, 2048]; partition p covers image rows [4p, 4p+4) (contiguous);
    # dim order matches the SBUF tile view [p, c, f].
    xr = x.rearrange("b c (hp hs) w -> b hp c (hs w)", hp=P)
    yr = y.rearrange("b c (hp hs) w -> b hp c (hs w)", hp=P)

    f32 = mybir.dt.float32
    HALVES = 2                  # groups per image
    F2 = F // HALVES            # free elems per plane per group
    with tile.TileContext(nc) as tc, ExitStack() as ctx:
        in_pool = ctx.enter_context(tc.tile_pool(name="in", bufs=6))
        out_pool = ctx.enter_context(tc.tile_pool(name="out", bufs=4))
        t_pool = ctx.enter_context(tc.tile_pool(name="tmp", bufs=4))

        for g in range(B_PER_CORE * HALVES):
            b, h = divmod(g, HALVES)
            fsl = slice(h * F2, (h + 1) * F2)
            # 1.5 MB transfers, half an image each.  Loads on the SP HWDGE
            # ring, stores on the ACT ring: each ring is FIFO per issuing
            # engine, so stores waiting on compute must not block loads.
            # ACT computes the *final* op per plane, so its store push never
            # waits on another engine.
            it = in_pool.tile([P, C * F2], f32)
            nc.sync.dma_start(
                it[:].rearrange("p (c f) -> p c f", c=C), xr[b][:, :, fsl]
            )
            ot = out_pool.tile([P, C * F2], f32)
            for c in range(C):
                # out_c = a[c,i]*X_i + a[c,j]*X_j + a[c,c]*X_c, diagonal term
                # largest; (i, j) = off-diagonals with |a_i| <= |a_j|:
                #   t1    = X_i * (a[c,i]/a[c,j]) + X_j     (DVE stt)
                #   t2    = t1 * (a[c,j]/a[c,c]) + X_c      (DVE stt)
                #   out_c = t2 * a[c,c]                     (ACT, single-src)
                i, j = [d for d in range(C) if d != c]
                if abs(a[c, i]) > abs(a[c, j]):
                    i, j = j, i
                sl = lambda d: slice(d * F2, (d + 1) * F2)
                t1 = t_pool.tile([P, F2], f32)
                nc.vector.scalar_tensor_tensor(
                    t1[:], it[:, sl(i)], float(a[c, i] / a[c, j]), it[:, sl(j)],
                    mybir.AluOpType.mult, mybir.AluOpType.add,
                )
                t2 = t_pool.tile([P, F2], f32, tag="t2")
                nc.vector.scalar_tensor_tensor(
                    t2[:], t1[:], float(a[c, j] / a[c, c]), it[:, sl(c)],
                    mybir.AluOpType.mult, mybir.AluOpType.add,
                )
                nc.scalar.mul(ot[:, sl(c)], t2[:], float(a[c, c]))
                # Store each plane as soon as its final op lands: finer store
                # pipelining and a ~3x shorter end-of-kernel drain.
                nc.scalar.dma_start(yr[b][:, c, fsl], ot[:, sl(c)])

    nc.compile()
    return nc


_NC = None


def _get_nc():
    global _NC
    if _NC is None:
        _NC = build_nc()
    return _NC


def _in_maps(i_en):
    xs = np.ascontiguousarray(np.asarray(i_en, dtype=np.float32)).reshape(
        N_CORES, B_PER_CORE, C, H, W
    )
    return [{"x": xs[i]} for i in range(N_CORES)]


def kernel(i_co=None, i_en=None, **_):
    res = run_bass_kernel_spmd(_get_nc(), _in_maps(i_en), list(range(N_CORES)))
    return np.concatenate(
        [res.results[i]["y"] for i in range(N_CORES)], axis=0
    )


# revision 12
# speedup vs baseline: 1.1494x; 1.0719x over previous
"""JPEGBase (nn_JPEGBase_240518169043) Trainium2 kernel.

The reference computes rgb->yuv, *255, blockwise 8x8 DCT, blockwise IDCT
(compress() is identity), /255, yuv->rgb.  The orthonormal DCT/IDCT pair and
the *255 / /255 cancel exactly, so the remaining math is a per-pixel 3x3
color-matrix roundtrip A = yuv2rgb @ rgb2yuv applied along the channel dim
(float32 discrepancy vs. the reference's explicit DCT roundtrip is ~1.5e-7
relative).  i_co is unused by the reference.

Sharding: pure data parallelism - batch 32 -> 4 images per core across 8
cores.  Per core the kernel streams 4 images of [3,512,512] f32 through SBUF
([128,2048] per plane), computes the three output planes as weighted sums of
the three input planes (2 DVE scalar_tensor_tensor ops + 1 ACT scale per
output plane), and streams them back.  Memory-bound: ~25 MB of HBM traffic
per core.
"""

import numpy as np
from contextlib import ExitStack

import concourse.bass as bass  # noqa: F401  (engine namespaces live on nc)
import concourse.tile as tile
from concourse import bacc, mybir
from concourse.bass_utils import run_bass_kernel_spmd

N_CORES = 8
B_FULL = 32
B_PER_CORE = B_FULL // N_CORES  # 4
C = 3
H = 512
W = 512
P = 128               # SBUF partitions
F = (H * W) // P      # 2048 floats per partition per plane


def _color_matrix():
    # kornia rgb_to_yuv / yuv_to_rgb coefficient matrices, composed in f64.
    m = np.array(
        [[0.299, 0.587, 0.114],
         [-0.147, -0.289, 0.436],
         [0.615, -0.515, -0.100]], dtype=np.float64)
    n = np.array(
        [[1.0, 0.0, 1.14],
         [1.0, -0.396, -0.581],
         [1.0, 2.029, 0.0]], dtype=np.float64)
    return n @ m


def build_nc():
    """Build + compile the per-core Bass program (same program on all cores)."""
    a = _color_matrix()
    nc = bacc.Bacc(
        "TRN2", target_bir_lowering=False, debug=False, num_devices=N_CORES
    )
    x = nc.dram_tensor(
        "x", [B_PER_CORE, C, H, W], mybir.dt.float32, kind="ExternalInput"
    ).ap()
    y = nc.dram_tensor(
        "y", [B_PER_CORE, C, H, W], mybir.dt.float32, kind="ExternalOutput"
    ).ap()
    # [b, 128, c, 2048]; partition p covers image rows [4p, 4p+4) (contiguous);
    # dim order matches the SBUF tile view [p, c, f].
    xr = x.rearrange("b c (hp hs) w -> b hp c (hs w)", hp=P)
    yr = y.rearrange("b c (hp hs) w -> b hp c (hs w)", hp=P)

    f32 = mybir.dt.float32
    HALVES = 2                  # groups per image
    F2 = F // HALVES            # free elems per plane per group
    with tile.TileContext(nc) as tc, ExitStack() as ctx:
        in_pool = ctx.enter_context(tc.tile_pool(name="in", bufs=6))
        out_pool = ctx.enter_context(tc.tile_pool(name="out", bufs=4))
        t_pool = ctx.enter_context(tc.tile_pool(name="tmp", bufs=4))

        # Work list: (image, free-offset, free-width).  Mostly half-image
        # groups (1.5 MB); the last half is split into two quarters so the
        # end-of-kernel drain (last compute + last store) is half as long.
        groups = [(b, h * F2, F2) for b in range(B_PER_CORE) for h in range(HALVES)]
        groups = groups[:-1] + [
            (B_PER_CORE - 1, (HALVES - 1) * F2, F2 // 2),
            (B_PER_CORE - 1, (HALVES - 1) * F2 + F2 // 2, F2 // 2),
        ]

        for gi, (b, f0, fw) in enumerate(groups):
            fsl = slice(f0, f0 + fw)
            # Loads on the SP HWDGE ring, stores on the ACT ring: each ring
            # is FIFO per issuing engine, so stores waiting on compute must
            # not block loads.  ACT computes the *final* op per plane, so
            # its store push never waits on another engine.
            it = in_pool.tile([P, C * F2], f32)
            if gi == 0:
                # Split the first load per plane, ordered so the planes the
                # first stt needs arrive first -> compute starts ~2us earlier.
                for c in (2, 1, 0):
                    nc.sync.dma_start(
                        it[:, c * fw:(c + 1) * fw], xr[b][:, c, fsl]
                    )
            else:
                nc.sync.dma_start(
                    it[:, :C * fw].rearrange("p (c f) -> p c f", c=C),
                    xr[b][:, :, fsl],
                )
            ot = out_pool.tile([P, C * F2], f32)
            for c in range(C):
                # out_c = a[c,i]*X_i + a[c,j]*X_j + a[c,c]*X_c, diagonal term
                # largest; (i, j) = off-diagonals with |a_i| <= |a_j|:
                #   t1    = X_i * (a[c,i]/a[c,j]) + X_j     (DVE stt)
                #   t2    = t1 * (a[c,j]/a[c,c]) + X_c      (DVE stt)
                #   out_c = t2 * a[c,c]                     (ACT, single-src)
                i, j = [d for d in range(C) if d != c]
                if abs(a[c, i]) > abs(a[c, j]):
                    i, j = j, i
                sl = lambda d: slice(d * fw, d * fw + fw)
                t1 = t_pool.tile([P, F2], f32)
                nc.vector.scalar_tensor_tensor(
                    t1[:, :fw], it[:, sl(i)], float(a[c, i] / a[c, j]),
                    it[:, sl(j)],
                    mybir.AluOpType.mult, mybir.AluOpType.add,
                )
                t2 = t_pool.tile([P, F2], f32, tag="t2")
                nc.vector.scalar_tensor_tensor(
                    t2[:, :fw], t1[:, :fw], float(a[c, j] / a[c, c]),
                    it[:, sl(c)],
                    mybir.AluOpType.mult, mybir.AluOpType.add,
                )
                nc.scalar.mul(ot[:, sl(c)], t2[:, :fw], float(a[c, c]))
            nc.scalar.dma_start(
                yr[b][:, :, fsl],
                ot[:, :C * fw].rearrange("p (c f) -> p c f", c=C),
            )

    nc.compile()
    return nc


_NC = None


def _get_nc():
    global _NC
    if _NC is None:
        _NC = build_nc()
    return _NC


def _in_maps(i_en):
    xs = np.ascontiguousarray(np.asarray(i_en, dtype=np.float32)).reshape(
        N_CORES, B_PER_CORE, C, H, W
    )
    return [{"x": xs[i]} for i in range(N_CORES)]


def kernel(i_co=None, i_en=None, **_):
    res = run_bass_kernel_spmd(_get_nc(), _in_maps(i_en), list(range(N_CORES)))
    return np.concatenate(
        [res.results[i]["y"] for i in range(N_CORES)], axis=0
    )
